# revision 17
# baseline (speedup 1.0000x reference)
"""Self-contained Trainium2 Bass kernel for a 2-layer GCN encoder (8 cores).

reference semantics (PyG GCNConv x2):
    out = Ahat @ relu(Ahat @ x @ W1 + b1) @ W2 + b2
    Ahat = D^-1/2 (A + I) D^-1/2,  deg = dst-counts + self-loops.

Strategy (graph/node parallel over 8 NeuronCores):
  * aggregation is linear => aggregate raw features first, matmul after:
        out_l = relu( D^-1/2 agg( D^-1/2 x ) @ W + b )
  * nodes are degree-sorted, packed into 128-node tiles, tiles dealt
    round-robin to the 8 cores; identical round structure across cores.
  * edges are flat per-(tile, table-chunk) lists (4 chunks keep dma_gather's
    int16 indices in range). One dma_gather instruction moves up to KG
    rounds of 128 neighbor rows; per round a one-hot assignment matrix A
    (built on DVE via tensor_scalar is_equal against an iota row) scatters
    the 128 gathered rows into the tile's 128 node slots via PE matmul
    accumulation in PSUM.
  * epilogue per tile: PSUM->SBUF, transposes + weight matmuls,
    per-partition dinv scale + bias, ReLU (layer 1).
  * one AllGather of the (optionally W2-premultiplied) hidden features
    between layers.
"""

import os
import sys
import numpy as np

for _p in ("/opt/trn_rl_repo",):
    if _p not in sys.path and os.path.isdir(_p):
        sys.path.insert(0, _p)

P = 128
NCH = 4
TG = 4


class Cfg:
    def __init__(self, N=100000, E=3200000, F_IN=256, F_HID=256, F_OUT=128,
                 C=8, gather_bf16=True, KG=32, w2first=True):
        self.N, self.E = N, E
        self.F_IN, self.F_HID, self.F_OUT = F_IN, F_HID, F_OUT
        self.C = C
        self.gather_bf16 = gather_bf16
        self.KG = KG
        self.w2first = w2first
        nt = (N + P) // P
        nt = ((nt + C - 1) // C) * C
        self.TPC = nt // C
        self.NT = nt
        self.NPAD = nt * P
        self.NPADL = self.TPC * P
        self.CH = self.NPAD // NCH
        assert self.NPAD > self.N
        assert self.CH <= 32768


def _prep(cfg, x, edge_index):
    import ml_dtypes
    N, C, TPC, CH = cfg.N, cfg.C, cfg.TPC, cfg.CH
    src = np.asarray(edge_index[0], dtype=np.int64)
    dst = np.asarray(edge_index[1], dtype=np.int64)
    deg = np.bincount(dst, minlength=N).astype(np.int64) + 1
    dinv = (1.0 / np.sqrt(deg)).astype(np.float32)

    order = np.argsort(-deg, kind="stable")
    i = np.arange(N)
    g_tile = i // P
    core_of = np.empty(N, np.int64)
    slot_of = np.empty(N, np.int64)
    part_of = np.empty(N, np.int64)
    core_of[order] = g_tile % C
    slot_of[order] = g_tile // C
    part_of[order] = i % P
    pad_id = (core_of * cfg.NPADL + slot_of * P + part_of).astype(np.int64)

    s_all = np.concatenate([src, np.arange(N, dtype=np.int64)])
    d_all = np.concatenate([dst, np.arange(N, dtype=np.int64)])

    ecore = core_of[d_all]
    eslot = slot_of[d_all]
    epart = part_of[d_all]
    esrc = pad_id[s_all]
    echunk = esrc // CH
    esrcloc = esrc % CH

    # per-(core, tile, chunk) edge counts; common (max) round counts
    keyc = (ecore * TPC + eslot) * NCH + echunk
    lens = np.bincount(keyc, minlength=C * TPC * NCH).reshape(C, TPC, NCH)
    rounds = -(-lens // 128)          # ceil
    rounds = rounds.max(axis=0)       # [TPC, NCH] shared by all cores
    R2 = rounds.sum(axis=1)           # rounds per tile
    R2SUM = int(rounds.sum())
    TOT = R2SUM * 128

    # segment base (in rounds) per (tile, chunk), ordered
    # group-major: g -> chunk -> tile-within-group
    seg_base = np.zeros((TPC, NCH), np.int64)
    acc = 0
    NG = -(-TPC // TG)
    for g in range(NG):
        ts = range(g * TG, min((g + 1) * TG, TPC))
        for h in range(NCH):
            for s in ts:
                seg_base[s, h] = acc
                acc += rounds[s, h]
    assert acc == R2SUM

    # place edges: per (core, tile, chunk) group, rank within group
    ordk = np.argsort(keyc, kind="stable")
    ksort = keyc[ordk]
    starts = np.searchsorted(ksort, np.arange(C * TPC * NCH), side="left")
    grp_of = ksort  # key per sorted edge
    rank = np.arange(len(ksort)) - starts[grp_of]
    e_core = grp_of // (TPC * NCH)
    e_slot = (grp_of // NCH) % TPC
    e_chunk = grp_of % NCH
    pos = seg_base[e_slot, e_chunk] * 128 + rank

    srcloc_flat = np.zeros((C, TOT), np.int16)
    slot_flat = np.full((C, TOT), 255, np.int64)
    srcloc_flat[e_core, pos] = esrcloc[ordk].astype(np.int16)
    slot_flat[e_core, pos] = epart[ordk]

    # wrapped int16 index layout: flat pos t -> (t % 16, t // 16), x8 groups
    idx16 = srcloc_flat.reshape(C, TOT // 16, 16).transpose(0, 2, 1)
    idx16 = np.tile(idx16, (1, 8, 1)).copy()
    # slot table: [core, lane, round] f32
    slots_tab = slot_flat.reshape(C, R2SUM, 128).transpose(0, 2, 1)
    slots_tab = np.ascontiguousarray(slots_tab, dtype=np.float32)

    dinv_pad = np.zeros(cfg.NPAD, np.float32)
    dinv_pad[pad_id] = dinv
    dinv_tabs = dinv_pad.reshape(C, TPC, P).transpose(0, 2, 1).copy()

    gnp = ml_dtypes.bfloat16 if cfg.gather_bf16 else np.float32
    xs_pad = np.zeros((cfg.NPAD, cfg.F_IN), np.float32)
    xs_pad[pad_id] = np.asarray(x, np.float32) * dinv[:, None]
    xs_pad = xs_pad.astype(gnp)

    iota = np.broadcast_to(
        np.arange(P, dtype=np.float32)[None, :], (P, P)).astype(gnp).copy()

    return dict(rounds=rounds, R2=R2, R2SUM=R2SUM, TOT=TOT,
                idx16=idx16, slots_tab=slots_tab, iota=iota,
                dinv_tabs=dinv_tabs, xs_pad=xs_pad, core_of=core_of,
                slot_of=slot_of, part_of=part_of)


def _build(cfg, prep):
    import concourse.bass as bass  # noqa: F401
    import concourse.bacc as bacc
    import concourse.mybir as mybir
    import concourse.tile as tile

    rounds, R2, R2SUM, TOT = (prep["rounds"], prep["R2"], prep["R2SUM"],
                              prep["TOT"])
    f32 = mybir.dt.float32
    i16 = mybir.dt.int16
    gdt = mybir.dt.bfloat16 if cfg.gather_bf16 else f32
    TPC, CH, KG = cfg.TPC, cfg.CH, cfg.KG
    F, FH, FO = cfg.F_IN, cfg.F_HID, cfg.F_OUT
    NG = -(-TPC // TG)

    nc = bacc.Bacc("TRN2", target_bir_lowering=False, debug=False,
                   enable_asserts=False, num_devices=cfg.C,
                   num_swdge_queues=4)

    xs_t = nc.dram_tensor("xs", [cfg.NPAD, F], gdt, kind="ExternalInput")
    idx_t = nc.dram_tensor("idx16", [P, TOT // 16], i16, kind="ExternalInput")
    slots_t = nc.dram_tensor("slots", [P, R2SUM], f32, kind="ExternalInput")
    iota_t = nc.dram_tensor("iota", [P, P], gdt, kind="ExternalInput")
    dinv_t = nc.dram_tensor("dinv", [P, TPC], f32, kind="ExternalInput")
    w1_t = nc.dram_tensor("w1", [F, FH], f32, kind="ExternalInput")
    b1_t = nc.dram_tensor("b1r", [P, FH], f32, kind="ExternalInput")
    w2_t = nc.dram_tensor("w2", [FH, FO], f32, kind="ExternalInput")
    b2_t = nc.dram_tensor("b2r", [P, FO], f32, kind="ExternalInput")
    ident_t = nc.dram_tensor("identf", [P, P], f32, kind="ExternalInput")
    out_t = nc.dram_tensor("out", [cfg.NPADL, FO], f32, kind="ExternalOutput")
    CW = FO if cfg.w2first else FH
    cc_in = nc.dram_tensor("cc_in", [cfg.NPADL, CW], gdt)
    cc_out = nc.dram_tensor("cc_out", [cfg.NPAD, CW], gdt, addr_space="Shared")

    # instruction plan per group: list of (chunk, [(tile, nrounds)...]) runs
    # split to <= KG rounds per dma_gather
    def plan_group(g):
        ts = list(range(g * TG, min((g + 1) * TG, TPC)))
        out = []
        for h in range(NCH):
            runs = []
            cur = []
            cur_n = 0
            for s in ts:
                r = int(rounds[s, h])
                while r > 0:
                    take = min(r, KG - cur_n)
                    if take > 0:
                        cur.append((s, take))
                        cur_n += take
                        r -= take
                    if cur_n == KG:
                        runs.append(cur)
                        cur, cur_n = [], 0
            if cur:
                runs.append(cur)
            out.append((h, runs))
        return ts, out

    with tile.TileContext(nc) as tc:
        with (
            tc.tile_pool(name="persist", bufs=1) as pp,
            tc.tile_pool(name="g", bufs=3) as gp,
            tc.tile_pool(name="ix", bufs=3) as ixp,
            tc.tile_pool(name="ap", bufs=6) as apl,
            tc.tile_pool(name="ep", bufs=3) as ep,
            tc.tile_pool(name="psA", bufs=TG, space="PSUM") as psA,
            tc.tile_pool(name="psT", bufs=2, space="PSUM") as psT,
            tc.tile_pool(name="psB", bufs=1, space="PSUM") as psB,
        ):
            slots_all = pp.tile([P, R2SUM], f32, tag="slots")
            nc.sync.dma_start(out=slots_all[:], in_=slots_t[:, :])
            dinv_all = pp.tile([P, TPC], f32, tag="dinv")
            nc.sync.dma_start(out=dinv_all[:], in_=dinv_t[:, :])
            ident = pp.tile([P, P], f32, tag="ident")
            nc.sync.dma_start(out=ident[:], in_=ident_t[:, :])
            iota_sb = pp.tile([P, P], gdt, tag="iota")
            nc.sync.dma_start(out=iota_sb[:], in_=iota_t[:, :])
            w_sb = {}
            for nm, wt, fo in (("w1", w1_t, FH), ("w2", w2_t, FO)):
                lst = []
                for k in range(2):
                    w = pp.tile([P, fo], f32, tag=f"{nm}_{k}")
                    nc.sync.dma_start(out=w[:], in_=wt[k * P:(k + 1) * P, :])
                    lst.append(w)
                w_sb[nm] = lst
            b1_sb = pp.tile([P, FH], f32, tag="b1")
            nc.sync.dma_start(out=b1_sb[:], in_=b1_t[:, :])
            b2_sb = pp.tile([P, FO], f32, tag="b2")
            nc.sync.dma_start(out=b2_sb[:], in_=b2_t[:, :])

            qctr = [0]

            def mm_apply(src_sb, wl, Fout, nchunk):
                """out = src_sb @ W via PE transpose + matmul; returns PSUM."""
                psum_hf = psB.tile([P, F], f32, tag="h")
                psum_h = psum_hf[:, :Fout]
                for k2 in range(nchunk):
                    psum_tt = psT.tile([P, P], f32, tag="tt")
                    nc.tensor.transpose(psum_tt[:],
                                        src_sb[:, k2 * P:(k2 + 1) * P],
                                        ident[:])
                    aggT = ep.tile([P, P], f32, tag="aggT")
                    nc.vector.tensor_copy(aggT[:], psum_tt[:])
                    nc.tensor.matmul(psum_h, lhsT=aggT[:],
                                     rhs=wl[k2][:], start=(k2 == 0),
                                     stop=(k2 == nchunk - 1))
                return psum_h

            def epilogue(s, psum_agg, first):
                if first:
                    agg_s = ep.tile([P, F], f32, tag="aggs")
                    nc.scalar.copy(agg_s[:], psum_agg)
                    psum_h = mm_apply(agg_s, w_sb["w1"], FH, F // P)
                    t1 = ep.tile([P, FH], f32, tag="t1")
                    nc.vector.tensor_scalar_mul(t1[:], psum_h,
                                                dinv_all[:, s:s + 1])
                    t2 = ep.tile([P, FH], f32, tag="t2")
                    nc.vector.tensor_add(t2[:], t1[:], b1_sb[:])
                    if cfg.w2first:
                        hs = ep.tile([P, FH], f32, tag="hs32")
                        nc.scalar.activation(
                            hs[:], t2[:], mybir.ActivationFunctionType.Relu,
                            scale=dinv_all[:, s:s + 1])
                        psum_z = mm_apply(hs, w_sb["w2"], FO, FH // P)
                        zs = ep.tile([P, FO], gdt, tag="zs")
                        nc.scalar.copy(zs[:], psum_z)
                        nc.sync.dma_start(
                            out=cc_in[s * P:(s + 1) * P, :], in_=zs[:])
                    else:
                        hs = ep.tile([P, FH], gdt, tag="hs")
                        nc.scalar.activation(
                            hs[:], t2[:], mybir.ActivationFunctionType.Relu,
                            scale=dinv_all[:, s:s + 1])
                        nc.sync.dma_start(
                            out=cc_in[s * P:(s + 1) * P, :], in_=hs[:])
                else:
                    if cfg.w2first:
                        t1 = ep.tile([P, FO], f32, tag="o1")
                        nc.vector.tensor_scalar_mul(
                            t1[:], psum_agg, dinv_all[:, s:s + 1])
                    else:
                        agg_s = ep.tile([P, F], f32, tag="aggs")
                        nc.scalar.copy(agg_s[:], psum_agg)
                        psum_h = mm_apply(agg_s, w_sb["w2"], FO, F // P)
                        t1 = ep.tile([P, FO], f32, tag="o1")
                        nc.vector.tensor_scalar_mul(
                            t1[:], psum_h, dinv_all[:, s:s + 1])
                    t2 = ep.tile([P, FO], f32, tag="o2")
                    nc.vector.tensor_add(t2[:], t1[:], b2_sb[:])
                    nc.sync.dma_start(out=out_t[s * P:(s + 1) * P, :],
                                      in_=t2[:])

            MAXGC = max(
                int(sum(rounds[s, h] for s in range(g * TG,
                                                    min((g + 1) * TG, TPC))
                        for h in range(NCH))) * 8
                for g in range(NG))

            def layer(table_t, Fa, first):
                t_round = 0
                t_idx = 0
                for g in range(NG):
                    ts, chunk_runs = plan_group(g)
                    psum = {}
                    cnt = {}
                    for s in ts:
                        pt = psA.tile([P, F], f32, tag="agg")
                        psum[s] = pt[:, :Fa]
                        cnt[s] = 0
                    # group idx block
                    gcols = int(sum(rounds[s, h] for s in ts
                                    for h in range(NCH))) * 8
                    ixg = ixp.tile([P, MAXGC], i16, tag="ixg")
                    nc.sync.dma_start(
                        out=ixg[:, :gcols],
                        in_=idx_t[:, t_idx // 16:t_idx // 16 + gcols])
                    gbase = t_idx
                    for h, runs in chunk_runs:
                        for run in runs:
                            nr = sum(t[1] for t in run)
                            n = nr * 128
                            gt = gp.tile([P, KG * Fa], gdt, tag=f"g{Fa}")
                            gv = gt[:, :].rearrange(
                                "p (j f) -> p j f", f=Fa)[:, :nr, :]
                            c0 = (t_idx - gbase) // 16
                            nc.gpsimd.dma_gather(
                                gv, table_t[h * CH:(h + 1) * CH, :],
                                ixg[:, c0:c0 + n // 16], n, n, Fa,
                                single_packet=False,
                                queue_num=qctr[0] % 4)
                            qctr[0] += 1
                            j = 0
                            for s, take in run:
                                for _ in range(take):
                                    A = apl.tile([P, P], gdt, tag="A")
                                    nc.vector.tensor_scalar(
                                        A[:], iota_sb[:],
                                        slots_all[:,
                                                  t_round + j:t_round + j + 1],
                                        None, mybir.AluOpType.is_equal)
                                    nc.tensor.matmul(
                                        psum[s], lhsT=A[:],
                                        rhs=gt[:, j * Fa:(j + 1) * Fa],
                                        start=(cnt[s] == 0),
                                        stop=(cnt[s] == int(R2[s]) - 1))
                                    cnt[s] += 1
                                    j += 1
                            t_idx += n
                            t_round += nr
                    for s in ts:
                        assert cnt[s] == int(R2[s])
                        epilogue(s, psum[s], first)

            layer(xs_t, F, first=True)
            nc.gpsimd.collective_compute(
                "AllGather", mybir.AluOpType.bypass,
                replica_groups=[list(range(cfg.C))],
                ins=[cc_in.ap().opt()], outs=[cc_out.ap().opt()],
            )
            layer(cc_out, CW, first=False)

    nc.compile()
    return nc, None


def _run(cfg, nc, prep, W1, b1, W2, b2, trace=False):
    from concourse.bass_utils import run_bass_kernel_spmd
    b1r = np.broadcast_to(np.asarray(b1, np.float32), (P, cfg.F_HID)).copy()
    b2r = np.broadcast_to(np.asarray(b2, np.float32), (P, cfg.F_OUT)).copy()
    in_maps = []
    for c in range(cfg.C):
        in_maps.append({
            "xs": prep["xs_pad"],
            "idx16": prep["idx16"][c],
            "slots": prep["slots_tab"][c],
            "iota": prep["iota"],
            "dinv": prep["dinv_tabs"][c],
            "w1": np.asarray(W1, np.float32),
            "b1r": b1r,
            "w2": np.asarray(W2, np.float32),
            "b2r": b2r,
            "identf": np.eye(P, dtype=np.float32),
        })
    res = run_bass_kernel_spmd(nc, in_maps, list(range(cfg.C)), trace=trace)
    outs = np.stack([res.results[c]["out"] for c in range(cfg.C)])
    out_full = np.empty((cfg.N, cfg.F_OUT), np.float32)
    co, so, po = prep["core_of"], prep["slot_of"], prep["part_of"]
    out_full[:] = outs[co, so * P + po]
    return out_full, res


def kernel(x, edge_index, W1, b1, W2, b2):
    cfg = Cfg(gather_bf16=bool(int(os.environ.get("GCN_BF16", "1"))),
              KG=int(os.environ.get("GCN_KG", "32")),
              w2first=bool(int(os.environ.get("GCN_W2F", "1"))))
    prep = _prep(cfg, x, edge_index)
    nc, _ = _build(cfg, prep)
    out, _ = _run(cfg, nc, prep, W1, b1, W2, b2,
                  trace=bool(int(os.environ.get("GCN_TRACE", "0"))))
    return out


# revision 18
# speedup vs baseline: 1.1896x; 1.1896x over previous
"""Self-contained Trainium2 Bass kernel for a 2-layer GCN encoder (8 cores).

reference semantics (PyG GCNConv x2):
    out = Ahat @ relu(Ahat @ x @ W1 + b1) @ W2 + b2
    Ahat = D^-1/2 (A + I) D^-1/2,  deg = dst-counts + self-loops.

Strategy (graph/node parallel over 8 NeuronCores):
  * aggregation is linear => aggregate raw features first, matmul after:
        out_l = relu( D^-1/2 agg( D^-1/2 x ) @ W + b )
  * nodes are degree-sorted, packed into 128-node tiles, tiles dealt
    round-robin to the 8 cores; identical round structure across cores.
  * edges are flat per-(tile, table-chunk) lists (4 chunks keep dma_gather's
    int16 indices in range). One dma_gather instruction moves up to KG
    rounds of 128 neighbor rows; per round a one-hot assignment matrix A
    (built on DVE via tensor_scalar is_equal against an iota row) scatters
    the 128 gathered rows into the tile's 128 node slots via PE matmul
    accumulation in PSUM.
  * epilogue per tile: PSUM->SBUF, transposes + weight matmuls,
    per-partition dinv scale + bias, ReLU (layer 1).
  * one AllGather of the (optionally W2-premultiplied) hidden features
    between layers.
"""

import os
import sys
import numpy as np

for _p in ("/opt/trn_rl_repo",):
    if _p not in sys.path and os.path.isdir(_p):
        sys.path.insert(0, _p)

P = 128
NCH = 4
TG = 4


class Cfg:
    def __init__(self, N=100000, E=3200000, F_IN=256, F_HID=256, F_OUT=128,
                 C=8, gather_bf16=True, KG=24, w2first=True):
        self.N, self.E = N, E
        self.F_IN, self.F_HID, self.F_OUT = F_IN, F_HID, F_OUT
        self.C = C
        self.gather_bf16 = gather_bf16
        self.KG = KG
        self.w2first = w2first
        nt = (N + P) // P
        nt = ((nt + C - 1) // C) * C
        self.TPC = nt // C
        self.NT = nt
        self.NPAD = nt * P
        self.NPADL = self.TPC * P
        self.CH = self.NPAD // NCH
        assert self.NPAD > self.N
        assert self.CH <= 32768


def _prep(cfg, x, edge_index):
    import ml_dtypes
    N, C, TPC, CH = cfg.N, cfg.C, cfg.TPC, cfg.CH
    src = np.asarray(edge_index[0], dtype=np.int64)
    dst = np.asarray(edge_index[1], dtype=np.int64)
    deg = np.bincount(dst, minlength=N).astype(np.int64) + 1
    dinv = (1.0 / np.sqrt(deg)).astype(np.float32)

    order = np.argsort(-deg, kind="stable")
    i = np.arange(N)
    g_tile = i // P
    core_of = np.empty(N, np.int64)
    slot_of = np.empty(N, np.int64)
    part_of = np.empty(N, np.int64)
    core_of[order] = g_tile % C
    slot_of[order] = g_tile // C
    part_of[order] = i % P
    pad_id = (core_of * cfg.NPADL + slot_of * P + part_of).astype(np.int64)

    s_all = np.concatenate([src, np.arange(N, dtype=np.int64)])
    d_all = np.concatenate([dst, np.arange(N, dtype=np.int64)])

    ecore = core_of[d_all]
    eslot = slot_of[d_all]
    epart = part_of[d_all]
    esrc = pad_id[s_all]
    echunk = esrc // CH
    esrcloc = esrc % CH

    # per-(core, tile, chunk) edge counts; common (max) round counts
    keyc = (ecore * TPC + eslot) * NCH + echunk
    lens = np.bincount(keyc, minlength=C * TPC * NCH).reshape(C, TPC, NCH)
    rounds = -(-lens // 128)          # ceil
    rounds = rounds.max(axis=0)       # [TPC, NCH] shared by all cores
    R2 = rounds.sum(axis=1)           # rounds per tile
    R2SUM = int(rounds.sum())
    TOT = R2SUM * 128

    # segment base (in rounds) per (tile, chunk), ordered
    # group-major: g -> chunk -> tile-within-group
    seg_base = np.zeros((TPC, NCH), np.int64)
    acc = 0
    NG = -(-TPC // TG)
    for g in range(NG):
        ts = range(g * TG, min((g + 1) * TG, TPC))
        for h in range(NCH):
            for s in ts:
                seg_base[s, h] = acc
                acc += rounds[s, h]
    assert acc == R2SUM

    # place edges: per (core, tile, chunk) group, rank within group
    ordk = np.lexsort((esrcloc, keyc))
    ksort = keyc[ordk]
    starts = np.searchsorted(ksort, np.arange(C * TPC * NCH), side="left")
    grp_of = ksort  # key per sorted edge
    rank = np.arange(len(ksort)) - starts[grp_of]
    e_core = grp_of // (TPC * NCH)
    e_slot = (grp_of // NCH) % TPC
    e_chunk = grp_of % NCH
    pos = seg_base[e_slot, e_chunk] * 128 + rank

    srcloc_flat = np.zeros((C, TOT), np.int16)
    slot_flat = np.full((C, TOT), 255, np.int64)
    srcloc_flat[e_core, pos] = esrcloc[ordk].astype(np.int16)
    slot_flat[e_core, pos] = epart[ordk]

    # wrapped int16 index layout: flat pos t -> (t % 16, t // 16), x8 groups
    idx16 = srcloc_flat.reshape(C, TOT // 16, 16).transpose(0, 2, 1)
    idx16 = np.tile(idx16, (1, 8, 1)).copy()
    # slot table: [core, lane, round] f32
    slots_tab = slot_flat.reshape(C, R2SUM, 128).transpose(0, 2, 1)
    slots_tab = np.ascontiguousarray(slots_tab, dtype=np.float32)

    dinv_pad = np.zeros(cfg.NPAD, np.float32)
    dinv_pad[pad_id] = dinv
    dinv_tabs = dinv_pad.reshape(C, TPC, P).transpose(0, 2, 1).copy()

    gnp = ml_dtypes.bfloat16 if cfg.gather_bf16 else np.float32
    xs_pad = np.zeros((cfg.NPAD, cfg.F_IN), np.float32)
    xs_pad[pad_id] = np.asarray(x, np.float32) * dinv[:, None]
    xs_pad = xs_pad.astype(gnp)

    iota = np.broadcast_to(
        np.arange(P, dtype=np.float32)[None, :], (P, P)).astype(gnp).copy()

    return dict(rounds=rounds, R2=R2, R2SUM=R2SUM, TOT=TOT,
                idx16=idx16, slots_tab=slots_tab, iota=iota,
                dinv_tabs=dinv_tabs, xs_pad=xs_pad, core_of=core_of,
                slot_of=slot_of, part_of=part_of)


def _build(cfg, prep):
    import concourse.bass as bass  # noqa: F401
    import concourse.bacc as bacc
    import concourse.mybir as mybir
    import concourse.tile as tile

    rounds, R2, R2SUM, TOT = (prep["rounds"], prep["R2"], prep["R2SUM"],
                              prep["TOT"])
    f32 = mybir.dt.float32
    i16 = mybir.dt.int16
    gdt = mybir.dt.bfloat16 if cfg.gather_bf16 else f32
    TPC, CH, KG = cfg.TPC, cfg.CH, cfg.KG
    F, FH, FO = cfg.F_IN, cfg.F_HID, cfg.F_OUT
    NG = -(-TPC // TG)

    nc = bacc.Bacc("TRN2", target_bir_lowering=False, debug=False,
                   enable_asserts=False, num_devices=cfg.C,
                   num_swdge_queues=4)

    xs_t = nc.dram_tensor("xs", [cfg.NPAD, F], gdt, kind="ExternalInput")
    idx_t = nc.dram_tensor("idx16", [P, TOT // 16], i16, kind="ExternalInput")
    slots_t = nc.dram_tensor("slots", [P, R2SUM], f32, kind="ExternalInput")
    iota_t = nc.dram_tensor("iota", [P, P], gdt, kind="ExternalInput")
    dinv_t = nc.dram_tensor("dinv", [P, TPC], f32, kind="ExternalInput")
    w1_t = nc.dram_tensor("w1", [F, FH], f32, kind="ExternalInput")
    b1_t = nc.dram_tensor("b1r", [P, FH], f32, kind="ExternalInput")
    w2_t = nc.dram_tensor("w2", [FH, FO], f32, kind="ExternalInput")
    b2_t = nc.dram_tensor("b2r", [P, FO], f32, kind="ExternalInput")
    ident_t = nc.dram_tensor("identf", [P, P], f32, kind="ExternalInput")
    out_t = nc.dram_tensor("out", [cfg.NPADL, FO], f32, kind="ExternalOutput")
    CW = FO if cfg.w2first else FH
    cc_in = nc.dram_tensor("cc_in", [cfg.NPADL, CW], gdt)
    cc_out = nc.dram_tensor("cc_out", [cfg.NPAD, CW], gdt, addr_space="Shared")

    # instruction plan per group: list of (chunk, [(tile, nrounds)...]) runs
    # split to <= KG rounds per dma_gather
    def plan_group(g):
        ts = list(range(g * TG, min((g + 1) * TG, TPC)))
        out = []
        for h in range(NCH):
            runs = []
            cur = []
            cur_n = 0
            for s in ts:
                r = int(rounds[s, h])
                while r > 0:
                    take = min(r, KG - cur_n)
                    if take > 0:
                        cur.append((s, take))
                        cur_n += take
                        r -= take
                    if cur_n == KG:
                        runs.append(cur)
                        cur, cur_n = [], 0
            if cur:
                runs.append(cur)
            out.append((h, runs))
        return ts, out

    with tile.TileContext(nc) as tc:
        with (
            tc.tile_pool(name="persist", bufs=1) as pp,
            tc.tile_pool(name="g", bufs=6) as gp,
            tc.tile_pool(name="ix", bufs=3) as ixp,
            tc.tile_pool(name="ap", bufs=12) as apl,
            tc.tile_pool(name="ep", bufs=3) as ep,
            tc.tile_pool(name="psA", bufs=TG, space="PSUM") as psA,
            tc.tile_pool(name="psT", bufs=2, space="PSUM") as psT,
            tc.tile_pool(name="psB", bufs=1, space="PSUM") as psB,
        ):
            slots_all = pp.tile([P, R2SUM], f32, tag="slots")
            nc.sync.dma_start(out=slots_all[:], in_=slots_t[:, :])
            dinv_all = pp.tile([P, TPC], f32, tag="dinv")
            nc.sync.dma_start(out=dinv_all[:], in_=dinv_t[:, :])
            ident = pp.tile([P, P], f32, tag="ident")
            nc.sync.dma_start(out=ident[:], in_=ident_t[:, :])
            iota_sb = pp.tile([P, P], gdt, tag="iota")
            nc.sync.dma_start(out=iota_sb[:], in_=iota_t[:, :])
            w_sb = {}
            for nm, wt, fo in (("w1", w1_t, FH), ("w2", w2_t, FO)):
                lst = []
                for k in range(2):
                    w = pp.tile([P, fo], f32, tag=f"{nm}_{k}")
                    nc.sync.dma_start(out=w[:], in_=wt[k * P:(k + 1) * P, :])
                    lst.append(w)
                w_sb[nm] = lst
            b1_sb = pp.tile([P, FH], f32, tag="b1")
            nc.sync.dma_start(out=b1_sb[:], in_=b1_t[:, :])
            b2_sb = pp.tile([P, FO], f32, tag="b2")
            nc.sync.dma_start(out=b2_sb[:], in_=b2_t[:, :])

            qctr = [0]

            def mm_apply(src_sb, wl, Fout, nchunk):
                """out = src_sb @ W via PE transpose + matmul; returns PSUM."""
                psum_hf = psB.tile([P, F], f32, tag="h")
                psum_h = psum_hf[:, :Fout]
                for k2 in range(nchunk):
                    psum_tt = psT.tile([P, P], f32, tag="tt")
                    nc.tensor.transpose(psum_tt[:],
                                        src_sb[:, k2 * P:(k2 + 1) * P],
                                        ident[:])
                    aggT = ep.tile([P, P], f32, tag="aggT")
                    nc.scalar.copy(aggT[:], psum_tt[:])
                    nc.tensor.matmul(psum_h, lhsT=aggT[:],
                                     rhs=wl[k2][:], start=(k2 == 0),
                                     stop=(k2 == nchunk - 1))
                return psum_h

            def epilogue(s, psum_agg, first):
                if first:
                    agg_s = ep.tile([P, F], f32, tag="aggs")
                    nc.scalar.copy(agg_s[:], psum_agg)
                    psum_h = mm_apply(agg_s, w_sb["w1"], FH, F // P)
                    t1 = ep.tile([P, FH], f32, tag="t1")
                    nc.vector.tensor_scalar_mul(t1[:], psum_h,
                                                dinv_all[:, s:s + 1])
                    t2 = ep.tile([P, FH], f32, tag="t2")
                    nc.vector.tensor_add(t2[:], t1[:], b1_sb[:])
                    if cfg.w2first:
                        hs = ep.tile([P, FH], f32, tag="hs32")
                        nc.scalar.activation(
                            hs[:], t2[:], mybir.ActivationFunctionType.Relu,
                            scale=dinv_all[:, s:s + 1])
                        psum_z = mm_apply(hs, w_sb["w2"], FO, FH // P)
                        zs = ep.tile([P, FO], gdt, tag="zs")
                        nc.scalar.copy(zs[:], psum_z)
                        nc.sync.dma_start(
                            out=cc_in[s * P:(s + 1) * P, :], in_=zs[:])
                    else:
                        hs = ep.tile([P, FH], gdt, tag="hs")
                        nc.scalar.activation(
                            hs[:], t2[:], mybir.ActivationFunctionType.Relu,
                            scale=dinv_all[:, s:s + 1])
                        nc.sync.dma_start(
                            out=cc_in[s * P:(s + 1) * P, :], in_=hs[:])
                else:
                    if cfg.w2first:
                        t1 = ep.tile([P, FO], f32, tag="o1")
                        nc.vector.tensor_scalar_mul(
                            t1[:], psum_agg, dinv_all[:, s:s + 1])
                    else:
                        agg_s = ep.tile([P, F], f32, tag="aggs")
                        nc.scalar.copy(agg_s[:], psum_agg)
                        psum_h = mm_apply(agg_s, w_sb["w2"], FO, F // P)
                        t1 = ep.tile([P, FO], f32, tag="o1")
                        nc.vector.tensor_scalar_mul(
                            t1[:], psum_h, dinv_all[:, s:s + 1])
                    t2 = ep.tile([P, FO], f32, tag="o2")
                    nc.vector.tensor_add(t2[:], t1[:], b2_sb[:])
                    nc.sync.dma_start(out=out_t[s * P:(s + 1) * P, :],
                                      in_=t2[:])

            MAXGC = max(
                int(sum(rounds[s, h] for s in range(g * TG,
                                                    min((g + 1) * TG, TPC))
                        for h in range(NCH))) * 8
                for g in range(NG))

            def layer(table_t, Fa, first):
                t_round = 0
                t_idx = 0
                for g in range(NG):
                    ts, chunk_runs = plan_group(g)
                    psum = {}
                    cnt = {}
                    for s in ts:
                        pt = psA.tile([P, F], f32, tag="agg")
                        psum[s] = pt[:, :Fa]
                        cnt[s] = 0
                    # group idx block
                    gcols = int(sum(rounds[s, h] for s in ts
                                    for h in range(NCH))) * 8
                    ixg = ixp.tile([P, MAXGC], i16, tag="ixg")
                    nc.sync.dma_start(
                        out=ixg[:, :gcols],
                        in_=idx_t[:, t_idx // 16:t_idx // 16 + gcols])
                    gbase = t_idx
                    for h, runs in chunk_runs:
                        for run in runs:
                            nr = sum(t[1] for t in run)
                            n = nr * 128
                            gt = gp.tile([P, KG * Fa], gdt, tag=f"g{Fa}")
                            gv = gt[:, :].rearrange(
                                "p (j f) -> p j f", f=Fa)[:, :nr, :]
                            c0 = (t_idx - gbase) // 16
                            nc.gpsimd.dma_gather(
                                gv, table_t[h * CH:(h + 1) * CH, :],
                                ixg[:, c0:c0 + n // 16], n, n, Fa,
                                single_packet=False,
                                queue_num=qctr[0] % 4)
                            qctr[0] += 1
                            j = 0
                            for s, take in run:
                                for _ in range(take):
                                    A = apl.tile([P, P], gdt, tag="A")
                                    nc.vector.tensor_scalar(
                                        A[:], iota_sb[:],
                                        slots_all[:,
                                                  t_round + j:t_round + j + 1],
                                        None, mybir.AluOpType.is_equal)
                                    nc.tensor.matmul(
                                        psum[s], lhsT=A[:],
                                        rhs=gt[:, j * Fa:(j + 1) * Fa],
                                        start=(cnt[s] == 0),
                                        stop=(cnt[s] == int(R2[s]) - 1))
                                    cnt[s] += 1
                                    j += 1
                            t_idx += n
                            t_round += nr
                    for s in ts:
                        assert cnt[s] == int(R2[s])
                        epilogue(s, psum[s], first)

            layer(xs_t, F, first=True)
            nc.gpsimd.collective_compute(
                "AllGather", mybir.AluOpType.bypass,
                replica_groups=[list(range(cfg.C))],
                ins=[cc_in.ap().opt()], outs=[cc_out.ap().opt()],
            )
            layer(cc_out, CW, first=False)

    nc.compile()
    return nc, None


def _run(cfg, nc, prep, W1, b1, W2, b2, trace=False):
    from concourse.bass_utils import run_bass_kernel_spmd
    b1r = np.broadcast_to(np.asarray(b1, np.float32), (P, cfg.F_HID)).copy()
    b2r = np.broadcast_to(np.asarray(b2, np.float32), (P, cfg.F_OUT)).copy()
    in_maps = []
    for c in range(cfg.C):
        in_maps.append({
            "xs": prep["xs_pad"],
            "idx16": prep["idx16"][c],
            "slots": prep["slots_tab"][c],
            "iota": prep["iota"],
            "dinv": prep["dinv_tabs"][c],
            "w1": np.asarray(W1, np.float32),
            "b1r": b1r,
            "w2": np.asarray(W2, np.float32),
            "b2r": b2r,
            "identf": np.eye(P, dtype=np.float32),
        })
    res = run_bass_kernel_spmd(nc, in_maps, list(range(cfg.C)), trace=trace)
    outs = np.stack([res.results[c]["out"] for c in range(cfg.C)])
    out_full = np.empty((cfg.N, cfg.F_OUT), np.float32)
    co, so, po = prep["core_of"], prep["slot_of"], prep["part_of"]
    out_full[:] = outs[co, so * P + po]
    return out_full, res


def kernel(x, edge_index, W1, b1, W2, b2):
    cfg = Cfg(gather_bf16=bool(int(os.environ.get("GCN_BF16", "1"))),
              KG=int(os.environ.get("GCN_KG", "24")),
              w2first=bool(int(os.environ.get("GCN_W2F", "1"))))
    prep = _prep(cfg, x, edge_index)
    nc, _ = _build(cfg, prep)
    out, _ = _run(cfg, nc, prep, W1, b1, W2, b2,
                  trace=bool(int(os.environ.get("GCN_TRACE", "0"))))
    return out


# revision 19
# speedup vs baseline: 1.8516x; 1.5565x over previous
"""Self-contained Trainium2 Bass kernel for a 2-layer GCN encoder (8 cores).

reference semantics (PyG GCNConv x2):
    out = Ahat @ relu(Ahat @ x @ W1 + b1) @ W2 + b2
    Ahat = D^-1/2 (A + I) D^-1/2,  deg = dst-counts + self-loops.

Strategy (graph/node parallel over 8 NeuronCores):
  * aggregation is linear => aggregate raw features first, matmul after:
        out_l = relu( D^-1/2 agg( D^-1/2 x ) @ W + b )
  * nodes are degree-sorted, packed into 128-node tiles, tiles dealt
    round-robin to the 8 cores; identical round structure across cores.
  * edges are flat per-(tile, table-chunk) lists (4 chunks keep dma_gather's
    int16 indices in range). One dma_gather instruction moves up to KG
    rounds of 128 neighbor rows; per round a one-hot assignment matrix A
    (built on DVE via tensor_scalar is_equal against an iota row) scatters
    the 128 gathered rows into the tile's 128 node slots via PE matmul
    accumulation in PSUM.
  * epilogue per tile: PSUM->SBUF, transposes + weight matmuls,
    per-partition dinv scale + bias, ReLU (layer 1).
  * one AllGather of the (optionally W2-premultiplied) hidden features
    between layers.
"""

import os
import sys
import numpy as np

for _p in ("/opt/trn_rl_repo",):
    if _p not in sys.path and os.path.isdir(_p):
        sys.path.insert(0, _p)

P = 128
NCH = 4
TG = 4


class Cfg:
    def __init__(self, N=100000, E=3200000, F_IN=256, F_HID=256, F_OUT=128,
                 C=8, gather_bf16=True, KG=24, w2first=True, astream=False):
        self.N, self.E = N, E
        self.F_IN, self.F_HID, self.F_OUT = F_IN, F_HID, F_OUT
        self.C = C
        self.gather_bf16 = gather_bf16
        self.KG = KG
        self.w2first = w2first
        self.astream = astream
        nt = (N + P) // P
        nt = ((nt + C - 1) // C) * C
        self.TPC = nt // C
        self.NT = nt
        self.NPAD = nt * P
        self.NPADL = self.TPC * P
        self.CH = self.NPAD // NCH
        assert self.NPAD > self.N
        assert self.CH <= 32768


def _prep(cfg, x, edge_index):
    import ml_dtypes
    N, C, TPC, CH = cfg.N, cfg.C, cfg.TPC, cfg.CH
    src = np.asarray(edge_index[0], dtype=np.int64)
    dst = np.asarray(edge_index[1], dtype=np.int64)
    deg = np.bincount(dst, minlength=N).astype(np.int64) + 1
    dinv = (1.0 / np.sqrt(deg)).astype(np.float32)

    order = np.argsort(-deg, kind="stable")
    i = np.arange(N)
    g_tile = i // P
    core_of = np.empty(N, np.int64)
    slot_of = np.empty(N, np.int64)
    part_of = np.empty(N, np.int64)
    core_of[order] = g_tile % C
    slot_of[order] = g_tile // C
    part_of[order] = i % P
    pad_id = (core_of * cfg.NPADL + slot_of * P + part_of).astype(np.int64)

    s_all = np.concatenate([src, np.arange(N, dtype=np.int64)])
    d_all = np.concatenate([dst, np.arange(N, dtype=np.int64)])

    ecore = core_of[d_all]
    eslot = slot_of[d_all]
    epart = part_of[d_all]
    esrc = pad_id[s_all]
    echunk = esrc // CH
    esrcloc = esrc % CH

    # per-(core, tile, chunk) edge counts; common (max) round counts
    keyc = (ecore * TPC + eslot) * NCH + echunk
    lens = np.bincount(keyc, minlength=C * TPC * NCH).reshape(C, TPC, NCH)
    rounds = -(-lens // 128)          # ceil
    rounds = rounds.max(axis=0)       # [TPC, NCH] shared by all cores
    R2 = rounds.sum(axis=1)           # rounds per tile
    R2SUM = int(rounds.sum())
    TOT = R2SUM * 128

    # segment base (in rounds) per (tile, chunk), ordered
    # group-major: g -> chunk -> tile-within-group
    seg_base = np.zeros((TPC, NCH), np.int64)
    acc = 0
    NG = -(-TPC // TG)
    for g in range(NG):
        ts = range(g * TG, min((g + 1) * TG, TPC))
        for h in range(NCH):
            for s in ts:
                seg_base[s, h] = acc
                acc += rounds[s, h]
    assert acc == R2SUM

    # place edges: per (core, tile, chunk) group, rank within group
    ordk = np.lexsort((esrcloc, keyc))
    ksort = keyc[ordk]
    starts = np.searchsorted(ksort, np.arange(C * TPC * NCH), side="left")
    grp_of = ksort  # key per sorted edge
    rank = np.arange(len(ksort)) - starts[grp_of]
    e_core = grp_of // (TPC * NCH)
    e_slot = (grp_of // NCH) % TPC
    e_chunk = grp_of % NCH
    pos = seg_base[e_slot, e_chunk] * 128 + rank

    srcloc_flat = np.zeros((C, TOT), np.int16)
    slot_flat = np.full((C, TOT), 255, np.int64)
    srcloc_flat[e_core, pos] = esrcloc[ordk].astype(np.int16)
    slot_flat[e_core, pos] = epart[ordk]

    # wrapped int16 index layout: flat pos t -> (t % 16, t // 16), x8 groups
    idx16 = srcloc_flat.reshape(C, TOT // 16, 16).transpose(0, 2, 1)
    idx16 = np.tile(idx16, (1, 8, 1)).copy()
    # slot table: [core, lane, round] f32
    slots_tab = slot_flat.reshape(C, R2SUM, 128).transpose(0, 2, 1)
    slots_tab = np.ascontiguousarray(slots_tab, dtype=np.float32)

    dinv_pad = np.zeros(cfg.NPAD, np.float32)
    dinv_pad[pad_id] = dinv
    dinv_tabs = dinv_pad.reshape(C, TPC, P).transpose(0, 2, 1).copy()

    gnp = ml_dtypes.bfloat16 if cfg.gather_bf16 else np.float32
    xs_pad = np.zeros((cfg.NPAD, cfg.F_IN), np.float32)
    xs_pad[pad_id] = np.asarray(x, np.float32) * dinv[:, None]
    xs_pad = xs_pad.astype(gnp)

    iota = np.broadcast_to(
        np.arange(P, dtype=np.float32)[None, :], (P, P)).astype(gnp).copy()

    a_tab = None
    if cfg.astream:
        sf = slot_flat.reshape(C, R2SUM, 128)
        oh = (sf[:, :, :, None] == np.arange(128, dtype=np.int64))
        a_tab = np.ascontiguousarray(
            oh.transpose(0, 2, 1, 3).reshape(C, P, TOT)).astype(gnp)

    return dict(rounds=rounds, R2=R2, R2SUM=R2SUM, TOT=TOT, a_tab=a_tab,
                idx16=idx16, slots_tab=slots_tab, iota=iota,
                dinv_tabs=dinv_tabs, xs_pad=xs_pad, core_of=core_of,
                slot_of=slot_of, part_of=part_of)


def _build(cfg, prep):
    import concourse.bass as bass  # noqa: F401
    import concourse.bacc as bacc
    import concourse.mybir as mybir
    import concourse.tile as tile

    rounds, R2, R2SUM, TOT = (prep["rounds"], prep["R2"], prep["R2SUM"],
                              prep["TOT"])
    f32 = mybir.dt.float32
    i16 = mybir.dt.int16
    gdt = mybir.dt.bfloat16 if cfg.gather_bf16 else f32
    TPC, CH, KG = cfg.TPC, cfg.CH, cfg.KG
    F, FH, FO = cfg.F_IN, cfg.F_HID, cfg.F_OUT
    NG = -(-TPC // TG)

    nc = bacc.Bacc("TRN2", target_bir_lowering=False, debug=False,
                   enable_asserts=False, num_devices=cfg.C,
                   num_swdge_queues=4)

    xs_t = nc.dram_tensor("xs", [cfg.NPAD, F], gdt, kind="ExternalInput")
    idx_t = nc.dram_tensor("idx16", [P, TOT // 16], i16, kind="ExternalInput")
    if cfg.astream:
        a_t = nc.dram_tensor("atab", [P, TOT], gdt, kind="ExternalInput")
    else:
        slots_t = nc.dram_tensor("slots", [P, R2SUM], f32,
                                 kind="ExternalInput")
        iota_t = nc.dram_tensor("iota", [P, P], gdt, kind="ExternalInput")
    dinv_t = nc.dram_tensor("dinv", [P, TPC], f32, kind="ExternalInput")
    w1_t = nc.dram_tensor("w1", [F, FH], f32, kind="ExternalInput")
    b1_t = nc.dram_tensor("b1r", [P, FH], f32, kind="ExternalInput")
    w2_t = nc.dram_tensor("w2", [FH, FO], f32, kind="ExternalInput")
    b2_t = nc.dram_tensor("b2r", [P, FO], f32, kind="ExternalInput")
    ident_t = nc.dram_tensor("identf", [P, P], f32, kind="ExternalInput")
    out_t = nc.dram_tensor("out", [cfg.NPADL, FO], f32, kind="ExternalOutput")
    CW = FO if cfg.w2first else FH
    cc_in = nc.dram_tensor("cc_in", [cfg.NPADL, CW], gdt)
    cc_out = nc.dram_tensor("cc_out", [cfg.NPAD, CW], gdt, addr_space="Shared")

    # instruction plan per group: list of (chunk, [(tile, nrounds)...]) runs
    # split to <= KG rounds per dma_gather
    def plan_group(g):
        ts = list(range(g * TG, min((g + 1) * TG, TPC)))
        out = []
        for h in range(NCH):
            runs = []
            cur = []
            cur_n = 0
            for s in ts:
                r = int(rounds[s, h])
                while r > 0:
                    take = min(r, KG - cur_n)
                    if take > 0:
                        cur.append((s, take))
                        cur_n += take
                        r -= take
                    if cur_n == KG:
                        runs.append(cur)
                        cur, cur_n = [], 0
            if cur:
                runs.append(cur)
            out.append((h, runs))
        return ts, out

    with tile.TileContext(nc) as tc:
        with (
            tc.tile_pool(name="persist", bufs=1) as pp,
            tc.tile_pool(name="g", bufs=(12 if cfg.KG <= 8 else 6)) as gp,
            tc.tile_pool(name="ix", bufs=3) as ixp,
            tc.tile_pool(name="ap", bufs=12) as apl,
            tc.tile_pool(name="ep", bufs=3) as ep,
            tc.tile_pool(name="psA", bufs=TG, space="PSUM") as psA,
            tc.tile_pool(name="psT", bufs=2, space="PSUM") as psT,
            tc.tile_pool(name="psB", bufs=1, space="PSUM") as psB,
        ):
            if not cfg.astream:
                slots_all = pp.tile([P, R2SUM], f32, tag="slots")
                nc.sync.dma_start(out=slots_all[:], in_=slots_t[:, :])
                iota_sb = pp.tile([P, P], gdt, tag="iota")
                nc.sync.dma_start(out=iota_sb[:], in_=iota_t[:, :])
            dinv_all = pp.tile([P, TPC], f32, tag="dinv")
            nc.sync.dma_start(out=dinv_all[:], in_=dinv_t[:, :])
            ident = pp.tile([P, P], f32, tag="ident")
            nc.sync.dma_start(out=ident[:], in_=ident_t[:, :])
            w_sb = {}
            for nm, wt, fo in (("w1", w1_t, FH), ("w2", w2_t, FO)):
                lst = []
                for k in range(2):
                    w = pp.tile([P, fo], f32, tag=f"{nm}_{k}")
                    nc.sync.dma_start(out=w[:], in_=wt[k * P:(k + 1) * P, :])
                    lst.append(w)
                w_sb[nm] = lst
            b1_sb = pp.tile([P, FH], f32, tag="b1")
            nc.sync.dma_start(out=b1_sb[:], in_=b1_t[:, :])
            b2_sb = pp.tile([P, FO], f32, tag="b2")
            nc.sync.dma_start(out=b2_sb[:], in_=b2_t[:, :])

            qctr = [0]

            def mm_apply(src_sb, wl, Fout, nchunk):
                """out = src_sb @ W via PE transpose + matmul; returns PSUM."""
                psum_hf = psB.tile([P, F], f32, tag="h")
                psum_h = psum_hf[:, :Fout]
                for k2 in range(nchunk):
                    psum_tt = psT.tile([P, P], f32, tag="tt")
                    nc.tensor.transpose(psum_tt[:],
                                        src_sb[:, k2 * P:(k2 + 1) * P],
                                        ident[:])
                    aggT = ep.tile([P, P], f32, tag="aggT")
                    nc.scalar.copy(aggT[:], psum_tt[:])
                    nc.tensor.matmul(psum_h, lhsT=aggT[:],
                                     rhs=wl[k2][:], start=(k2 == 0),
                                     stop=(k2 == nchunk - 1))
                return psum_h

            def epilogue(s, psum_agg, first):
                if first:
                    agg_s = ep.tile([P, F], f32, tag="aggs")
                    nc.scalar.copy(agg_s[:], psum_agg)
                    psum_h = mm_apply(agg_s, w_sb["w1"], FH, F // P)
                    t1 = ep.tile([P, FH], f32, tag="t1")
                    nc.vector.tensor_scalar_mul(t1[:], psum_h,
                                                dinv_all[:, s:s + 1])
                    t2 = ep.tile([P, FH], f32, tag="t2")
                    nc.vector.tensor_add(t2[:], t1[:], b1_sb[:])
                    if cfg.w2first:
                        hs = ep.tile([P, FH], f32, tag="hs32")
                        nc.scalar.activation(
                            hs[:], t2[:], mybir.ActivationFunctionType.Relu,
                            scale=dinv_all[:, s:s + 1])
                        psum_z = mm_apply(hs, w_sb["w2"], FO, FH // P)
                        zs = ep.tile([P, FO], gdt, tag="zs")
                        nc.scalar.copy(zs[:], psum_z)
                        nc.sync.dma_start(
                            out=cc_in[s * P:(s + 1) * P, :], in_=zs[:])
                    else:
                        hs = ep.tile([P, FH], gdt, tag="hs")
                        nc.scalar.activation(
                            hs[:], t2[:], mybir.ActivationFunctionType.Relu,
                            scale=dinv_all[:, s:s + 1])
                        nc.sync.dma_start(
                            out=cc_in[s * P:(s + 1) * P, :], in_=hs[:])
                else:
                    if cfg.w2first:
                        t1 = ep.tile([P, FO], f32, tag="o1")
                        nc.vector.tensor_scalar_mul(
                            t1[:], psum_agg, dinv_all[:, s:s + 1])
                    else:
                        agg_s = ep.tile([P, F], f32, tag="aggs")
                        nc.scalar.copy(agg_s[:], psum_agg)
                        psum_h = mm_apply(agg_s, w_sb["w2"], FO, F // P)
                        t1 = ep.tile([P, FO], f32, tag="o1")
                        nc.vector.tensor_scalar_mul(
                            t1[:], psum_h, dinv_all[:, s:s + 1])
                    t2 = ep.tile([P, FO], f32, tag="o2")
                    nc.vector.tensor_add(t2[:], t1[:], b2_sb[:])
                    nc.sync.dma_start(out=out_t[s * P:(s + 1) * P, :],
                                      in_=t2[:])

            MAXGC = max(
                int(sum(rounds[s, h] for s in range(g * TG,
                                                    min((g + 1) * TG, TPC))
                        for h in range(NCH))) * 8
                for g in range(NG))

            def layer(table_t, Fa, first):
                t_round = 0
                t_idx = 0
                for g in range(NG):
                    ts, chunk_runs = plan_group(g)
                    psum = {}
                    cnt = {}
                    for s in ts:
                        pt = psA.tile([P, F], f32, tag="agg")
                        psum[s] = pt[:, :Fa]
                        cnt[s] = 0
                    # group idx block
                    gcols = int(sum(rounds[s, h] for s in ts
                                    for h in range(NCH))) * 8
                    ixg = ixp.tile([P, MAXGC], i16, tag="ixg")
                    nc.sync.dma_start(
                        out=ixg[:, :gcols],
                        in_=idx_t[:, t_idx // 16:t_idx // 16 + gcols])
                    gbase = t_idx
                    for h, runs in chunk_runs:
                        for run in runs:
                            nr = sum(t[1] for t in run)
                            n = nr * 128
                            gt = gp.tile([P, KG * Fa], gdt, tag=f"g{Fa}")
                            gv = gt[:, :].rearrange(
                                "p (j f) -> p j f", f=Fa)[:, :nr, :]
                            c0 = (t_idx - gbase) // 16
                            nc.gpsimd.dma_gather(
                                gv, table_t[h * CH:(h + 1) * CH, :],
                                ixg[:, c0:c0 + n // 16], n, n, Fa,
                                single_packet=False,
                                queue_num=qctr[0] % 4)
                            qctr[0] += 1
                            if cfg.astream:
                                At = apl.tile([P, KG * P], gdt, tag="At")
                                nc.scalar.dma_start(
                                    out=At[:, :nr * P],
                                    in_=a_t[:, t_round * P:
                                            (t_round + nr) * P])
                            j = 0
                            for s, take in run:
                                for _ in range(take):
                                    if cfg.astream:
                                        Aj = At[:, j * P:(j + 1) * P]
                                    else:
                                        A = apl.tile([P, P], gdt, tag="A")
                                        nc.vector.tensor_scalar(
                                            A[:], iota_sb[:],
                                            slots_all[:, t_round + j:
                                                      t_round + j + 1],
                                            None, mybir.AluOpType.is_equal)
                                        Aj = A[:]
                                    nc.tensor.matmul(
                                        psum[s], lhsT=Aj,
                                        rhs=gt[:, j * Fa:(j + 1) * Fa],
                                        start=(cnt[s] == 0),
                                        stop=(cnt[s] == int(R2[s]) - 1))
                                    cnt[s] += 1
                                    j += 1
                            t_idx += n
                            t_round += nr
                    for s in ts:
                        assert cnt[s] == int(R2[s])
                        epilogue(s, psum[s], first)

            layer(xs_t, F, first=True)
            nc.gpsimd.collective_compute(
                "AllGather", mybir.AluOpType.bypass,
                replica_groups=[list(range(cfg.C))],
                ins=[cc_in.ap().opt()], outs=[cc_out.ap().opt()],
            )
            layer(cc_out, CW, first=False)

    nc.compile()
    return nc, None


def _run(cfg, nc, prep, W1, b1, W2, b2, trace=False):
    from concourse.bass_utils import run_bass_kernel_spmd
    b1r = np.broadcast_to(np.asarray(b1, np.float32), (P, cfg.F_HID)).copy()
    b2r = np.broadcast_to(np.asarray(b2, np.float32), (P, cfg.F_OUT)).copy()
    in_maps = []
    for c in range(cfg.C):
        im = {
            "xs": prep["xs_pad"],
            "idx16": prep["idx16"][c],
            "dinv": prep["dinv_tabs"][c],
            "w1": np.asarray(W1, np.float32),
            "b1r": b1r,
            "w2": np.asarray(W2, np.float32),
            "b2r": b2r,
            "identf": np.eye(P, dtype=np.float32),
        }
        if cfg.astream:
            im["atab"] = prep["a_tab"][c]
        else:
            im["slots"] = prep["slots_tab"][c]
            im["iota"] = prep["iota"]
        in_maps.append(im)
    res = run_bass_kernel_spmd(nc, in_maps, list(range(cfg.C)), trace=trace)
    outs = np.stack([res.results[c]["out"] for c in range(cfg.C)])
    out_full = np.empty((cfg.N, cfg.F_OUT), np.float32)
    co, so, po = prep["core_of"], prep["slot_of"], prep["part_of"]
    out_full[:] = outs[co, so * P + po]
    return out_full, res


def kernel(x, edge_index, W1, b1, W2, b2):
    cfg = Cfg(gather_bf16=bool(int(os.environ.get("GCN_BF16", "1"))),
              KG=int(os.environ.get("GCN_KG", "24")),
              w2first=bool(int(os.environ.get("GCN_W2F", "1"))),
              astream=bool(int(os.environ.get("GCN_AS", "0"))))
    prep = _prep(cfg, x, edge_index)
    nc, _ = _build(cfg, prep)
    out, _ = _run(cfg, nc, prep, W1, b1, W2, b2,
                  trace=bool(int(os.environ.get("GCN_TRACE", "0"))))
    return out
